# revision 24
# baseline (speedup 1.0000x reference)
"""GAT (2-layer graph attention network) Trainium2 kernel, 8-core SPMD.

Sharding: core c -> head-group hg=c//4 (4 of 8 heads) x query-block rg=c%4
(1024 of 4096 queries).  Layer-2 (single head) is computed redundantly by
head-group pairs so the SPMD program stays uniform (no core-dependent
addressing); host keeps cores 0-3's outputs.

Attention is computed in transposed [j(part), i(free)] layout so the value
matmul contracts j on the PE partition dim.  Two elementwise pipelines are
load-balanced across ACT and DVE:
  ACT pipe:  L = Lrelu(wh1_i + wh2_j); A = Exp(L); att = A * mask
  DVE pipe:  att = max(exp(wh1_i)*exp(wh2_j), exp(.2wh1_i)*exp(.2wh2_j)) * mask
(the second uses exp(leaky(t)) = max(exp(t), exp(0.2 t)) and rank-1 products).
Row-softmax denominators ride along as an extra all-ones column of the
stationary [wh | 1] operand; normalization happens on the small [64, i] output.
"""

import os
import sys

sys.path.insert(0, "/opt/trn_rl_repo")

import numpy as np

import concourse.bass as bass
import concourse.mybir as mybir
import concourse.tile as tile

N, FIN, H, HID, NCLS = 4096, 512, 8, 64, 64
NCORES = 8
HPC = H // 2          # heads per core (head-group size 4)
QC = 1024             # L1 queries per core
JT = N // 128         # 32 j-tiles
KT = FIN // 128       # 4 k-tiles
NT = N // 128         # 32 n-tiles
ALPHA = 0.2

F16 = mybir.dt.float16
F32 = mybir.dt.float32
AF = mybir.ActivationFunctionType
OP = mybir.AluOpType

# jt indices handled by the DVE-only pipeline (rest go to the ACT pipeline).
DVE_FRAC = float(os.environ.get("GAT_DVE_FRAC", "0.44"))
# "lrelu": ACT pipe = Lrelu -> Exp -> mask-mul (1 DVE op / tile)
# "dexp":  ACT pipe = Exp, Exp(0.2x) -> max -> mask-mul (2 DVE ops / tile);
#          avoids the Lrelu activation (not implemented in CoreSim).
ACT_MODE = os.environ.get("GAT_ACT_MODE", "lrelu")


def _dve_set(njt: int) -> set:
    k = round(njt * DVE_FRAC)
    if k <= 0:
        return set()
    stride = njt / k
    return {int(i * stride) for i in range(k)}




def _bcast_via_dram(nc, dram, out_ap, row_ap, parts, width, dt, tag):
    """Broadcast a [1, width] SBUF row across partitions via a DRAM bounce
    (SBUF sources cannot have partition-step-0 APs, DRAM sources can)."""
    d = dram.tile([1, width], dt, tag=tag, name=tag)
    nc.gpsimd.dma_start(d[:], row_ap)
    nc.gpsimd.dma_start(out_ap, d[:].to_broadcast((parts, width)))


def _attention_head(nc, pools, whO_fn, wh1_row, wh2_fn, mT_sb, out_fn):
    """One attention 'head': JT j-tiles over QC queries in [j, i] layout.

    whO_fn(jt) -> AP [128, 65] stationary [wh | ones] for tile jt
    wh1_row:  AP [1, QC] f32 (pre-activation wh1 for this core's queries)
    wh2_fn(jt) -> AP [128, 1] f32 wh2 col; wh2_fn(None) -> [128, JT] all cols
    out_fn(ih) -> destination AP [64, 512] for normalized+ELU output (transposed)
    """
    work, small, rows, bcast, psumV = (pools["work"], pools["small"],
                                       pools["rows"], pools["bcast"],
                                       pools["psumV"])

    # exp'd per-node vectors
    u_row = rows.tile([1, QC], F16, tag="u_row")
    p_row = rows.tile([1, QC], F16, tag="p_row")
    nc.scalar.activation(u_row[:], wh1_row, AF.Exp)
    nc.scalar.activation(p_row[:], wh1_row, AF.Exp, scale=ALPHA)
    v_cols = rows.tile([128, JT], F32, tag="v_cols")
    q_cols = rows.tile([128, JT], F32, tag="q_cols")
    wh2_all = wh2_fn(None)
    nc.scalar.activation(v_cols[:], wh2_all, AF.Exp)
    nc.scalar.activation(q_cols[:], wh2_all, AF.Exp, scale=ALPHA)
    b2_cols = None
    if ACT_MODE == "dexp":
        b2_cols = rows.tile([128, JT], F32, tag="b2_cols")
        nc.vector.tensor_scalar_mul(b2_cols[:], wh2_all, ALPHA)

    # partition-broadcast tiles
    w1bc = bcast.tile([128, QC], F32, tag="w1bc")
    u_bc = bcast.tile([128, QC], F16, tag="u_bc")
    p_bc = bcast.tile([128, QC], F16, tag="p_bc")
    dram = pools["dram"]
    _bcast_via_dram(nc, dram, w1bc[:], wh1_row, 128, QC, F32, "bc_w1")
    _bcast_via_dram(nc, dram, u_bc[:], u_row[:], 128, QC, F16, "bc_u")
    _bcast_via_dram(nc, dram, p_bc[:], p_row[:], 128, QC, F16, "bc_p")

    psV = [psumV.tile([65, 512], F32, tag=f"psV{ih}", name=f"psV{ih}")
           for ih in range(2)]
    dve_set = _dve_set(JT)
    for jt in range(JT):
        att = work.tile([128, QC], F16, tag="att")
        m_ap = mT_sb[:, jt * QC:(jt + 1) * QC]
        if jt in dve_set:
            uv = work.tile([128, QC], F16, tag="uv")
            pq = work.tile([128, QC], F16, tag="pq")
            nc.vector.tensor_scalar_mul(uv[:], u_bc[:], v_cols[:, jt:jt + 1])
            nc.vector.tensor_scalar_mul(pq[:], p_bc[:], q_cols[:, jt:jt + 1])
            nc.vector.tensor_max(att[:], uv[:], pq[:])
            nc.vector.tensor_mul(att[:], att[:], m_ap)
        elif ACT_MODE == "lrelu":
            lt = work.tile([128, QC], F32, tag="lt")
            at = work.tile([128, QC], F16, tag="at")
            nc.scalar.activation(lt[:], w1bc[:], AF.Lrelu,
                                 bias=wh2_fn(jt), alpha=ALPHA)
            nc.scalar.activation(at[:], lt[:], AF.Exp)
            nc.vector.tensor_mul(att[:], at[:], m_ap)
        else:
            aa = work.tile([128, QC], F16, tag="aa")
            bb = work.tile([128, QC], F16, tag="bb")
            nc.scalar.activation(aa[:], w1bc[:], AF.Exp, bias=wh2_fn(jt))
            nc.scalar.activation(bb[:], w1bc[:], AF.Exp,
                                 bias=b2_cols[:, jt:jt + 1], scale=ALPHA)
            nc.vector.tensor_max(att[:], aa[:], bb[:])
            nc.vector.tensor_mul(att[:], att[:], m_ap)
        for ih in range(2):
            nc.tensor.matmul(psV[ih][:], lhsT=whO_fn(jt),
                             rhs=att[:, ih * 512:(ih + 1) * 512],
                             start=(jt == 0), stop=(jt == JT - 1))

    # epilogue: normalize by the sums row (64), ELU, write transposed out
    for ih in range(2):
        srec = small.tile([1, 512], F32, tag="srec")
        nc.vector.reciprocal(srec[:], psV[ih][64:65, :])
        rsbc = small.tile([64, 512], F32, tag="rsbc")
        _bcast_via_dram(nc, pools["dram"], rsbc[:], srec[:], 64, 512, F32, "bc_s")
        onum = small.tile([64, 512], F32, tag="onum")
        nc.vector.tensor_mul(onum[:], psV[ih][0:64, :], rsbc[:])
        # ELU(x) = relu(x) + exp(min(x, 0)) - 1
        rr = small.tile([64, 512], F32, tag="rr")
        cm = small.tile([64, 512], F32, tag="cm")
        ec = small.tile([64, 512], F32, tag="ec")
        nc.vector.tensor_scalar_max(rr[:], onum[:], 0.0)
        nc.vector.tensor_scalar_min(cm[:], onum[:], 0.0)
        nc.scalar.activation(ec[:], cm[:], AF.Exp)
        nc.vector.tensor_add(rr[:], rr[:], ec[:])
        nc.vector.tensor_scalar_sub(out_fn(ih), rr[:], 1.0)


def _legalize_dma_waits(nc):
    """This toolchain's walrus allows at most ONE sync-wait per instruction.
    Tile freely assigns several.  Split the extras into single-wait
    EventSemaphore instructions inserted right before (each engine/queue
    sequencer executes its stream in program order, so ordering holds)."""
    import bass_rust
    nfix = 0
    for func in nc.m.functions:
        for blk in func.blocks:
            insts = blk.instructions
            new = []
            changed = False
            for inst in insts:
                si = getattr(inst, "sync_info", None)
                tname = type(inst).__name__
                if (tname != "InstEventSemaphore" and si is not None
                        and si.on_wait is not None and len(si.on_wait) > 1):
                    for wi, w in enumerate(si.on_wait[:-1]):
                        ev = mybir.InstEventSemaphore(
                            name=f"{inst.name}w{wi}", ins=[], outs=[])
                        ev.engine = inst.engine
                        ev.sync_info = bass_rust.SyncInfo(
                            on_wait=[w], on_update=[])
                        new.append(ev)
                    si.on_wait = si.on_wait[-1:]
                    nfix += 1
                    changed = True
                new.append(inst)
            if changed:
                blk.instructions = new
    return nfix


def build_program():
    nc = bass.Bass("TRN2", target_bir_lowering=False, debug=False,
                   num_devices=NCORES)

    xT = nc.dram_tensor("xT", [FIN, N], F16, kind="ExternalInput")
    xTq = nc.dram_tensor("xTq", [FIN, QC], F16, kind="ExternalInput")
    mT = nc.dram_tensor("mT", [N, QC], F16, kind="ExternalInput")
    wcat = nc.dram_tensor("wcat", [FIN, HPC * 66], F16, kind="ExternalInput")
    gb = nc.dram_tensor("gb", [FIN, HPC], F16, kind="ExternalInput")
    w2cat = nc.dram_tensor("w2cat", [FIN, 66], F16, kind="ExternalInput")
    g1l2m = nc.dram_tensor("g1l2m", [128, 2], F16, kind="ExternalInput")
    out = nc.dram_tensor("out", [NCLS, QC], F32, kind="ExternalOutput")

    with tile.TileContext(nc) as tc:
        import contextlib
        ctx = contextlib.ExitStack()
        with ctx:
            resid = ctx.enter_context(tc.tile_pool(name="resid", bufs=1))
            bcast = ctx.enter_context(tc.tile_pool(name="bcast", bufs=2))
            work = ctx.enter_context(tc.tile_pool(name="work", bufs=2))
            small = ctx.enter_context(tc.tile_pool(name="small", bufs=1))
            rows = ctx.enter_context(tc.tile_pool(name="rows", bufs=2))
            stream = ctx.enter_context(tc.tile_pool(name="stream", bufs=1))
            psumV = ctx.enter_context(
                tc.tile_pool(name="psumV", bufs=1, space="PSUM"))
            psumF = ctx.enter_context(
                tc.tile_pool(name="psumF", bufs=2, space="PSUM"))
            dram = ctx.enter_context(
                tc.tile_pool(name="dram", bufs=2, space="DRAM"))
            pools = {"work": work, "small": small, "rows": rows,
                     "bcast": bcast, "psumV": psumV, "dram": dram}

            # ---- resident tensors ----
            mT_sb = resid.tile([128, JT * QC], F16)  # 8 MiB
            for jt in range(JT):
                nc.sync.dma_start(mT_sb[:, jt * QC:(jt + 1) * QC],
                                  mT[jt * 128:(jt + 1) * 128, :])

            whO = resid.tile([128, HPC * NT * 65], F16)   # [wh | 1] layout L1
            wh12 = resid.tile([128, HPC * NT * 2], F32)   # wh1/wh2 columns
            nc.vector.memset(whO[:], 1.0)  # ones cols survive the wh copies
            w1rows = [resid.tile([1, QC], F32, tag=f"w1r{i}", name=f"w1r{i}")
                      for i in range(HPC)]
            hpatch = [resid.tile([128, QC], F16, tag=f"hp{i}", name=f"hp{i}")
                      for i in range(2)]
            whO2 = resid.tile([128, NT * 65], F16)
            nc.vector.memset(whO2[:], 1.0)
            wh2L2 = resid.tile([128, NT], F32)
            w1l2row = resid.tile([1, QC], F32)
            ones64 = resid.tile([64, 1], F32)
            nc.vector.memset(ones64[:], 1.0)
            w2cat_sb = resid.tile([128, KT * 66], F16)
            g1l2m_sb = resid.tile([128, 2], F16)
            for kt in range(KT):
                nc.sync.dma_start(w2cat_sb[:, kt * 66:(kt + 1) * 66],
                                  w2cat[kt * 128:(kt + 1) * 128, :])
            nc.sync.dma_start(g1l2m_sb[:], g1l2m[:, :])
            oe = resid.tile([64, QC], F32)
            e2 = resid.tile([64, QC], F32)
            lse = resid.tile([1, QC], F32)
            lsebc = resid.tile([64, QC], F16)

            whO_r = whO.rearrange("p (h n c) -> p h n c", h=HPC, c=65)
            wh12_r = wh12.rearrange("p (h n c) -> p h n c", h=HPC, c=2)

            # ---- stage F: feature matmuls (wh for this core's 4 heads) ----
            with tc.tile_pool(name="feat", bufs=1) as feat:
                wcat_sb = feat.tile([128, KT * HPC * 66], F16)
                gb_sb = feat.tile([128, KT * HPC], F16)
                for kt in range(KT):
                    nc.sync.dma_start(
                        wcat_sb[:, kt * HPC * 66:(kt + 1) * HPC * 66],
                        wcat[kt * 128:(kt + 1) * 128, :])
                    nc.sync.dma_start(gb_sb[:, kt * HPC:(kt + 1) * HPC],
                                      gb[kt * 128:(kt + 1) * 128, :])

                for nt in range(NT):
                    psF = psumF.tile([128, HPC * 66], F32, tag="fps")
                    for kt in range(KT):
                        xs = feat.tile([128, 128], F16, tag="xs", bufs=8)
                        nc.gpsimd.dma_start(
                            xs[:], xT[kt * 128:(kt + 1) * 128,
                                      nt * 128:(nt + 1) * 128])
                        nc.tensor.matmul(
                            psF[:], lhsT=xs[:],
                            rhs=wcat_sb[:, kt * HPC * 66:(kt + 1) * HPC * 66],
                            start=(kt == 0), stop=(kt == KT - 1))
                    psF_r = psF.rearrange("p (h c) -> p h c", c=66)
                    nc.vector.tensor_copy(whO_r[:, :, nt, 0:64],
                                          psF_r[:, :, 0:64])
                    nc.vector.tensor_copy(wh12_r[:, :, nt, :],
                                          psF_r[:, :, 64:66])

                # wh1 rows for this core's queries (one M=1 matmul per head
                # so every row tile starts at partition 0)
                for ih in range(2):
                    psw = [psumF.tile([1, 512], F32, tag=f"pswl{l}",
                                      name=f"pswl{l}", bufs=1)
                           for l in range(HPC)]
                    for kt in range(KT):
                        xq = feat.tile([128, 512], F16, tag="xq", bufs=4)
                        nc.gpsimd.dma_start(
                            xq[:], xTq[kt * 128:(kt + 1) * 128,
                                       ih * 512:(ih + 1) * 512])
                        for l in range(HPC):
                            nc.tensor.matmul(
                                psw[l][:],
                                lhsT=gb_sb[:, kt * HPC + l:kt * HPC + l + 1],
                                rhs=xq[:],
                                start=(kt == 0), stop=(kt == KT - 1))
                    for l in range(HPC):
                        nc.vector.tensor_copy(
                            w1rows[l][:, ih * 512:(ih + 1) * 512], psw[l][:])

            # ---- stage A: L1 attention, 4 heads ----
            for l in range(HPC):
                def whO_fn(jt, l=l):
                    return whO[:, (l * NT + jt) * 65:(l * NT + jt) * 65 + 65]

                def wh2_fn(jt, l=l):
                    if jt is None:
                        return wh12_r[:, l, :, 1]
                    return wh12[:, (l * NT + jt) * 2 + 1:(l * NT + jt) * 2 + 2]

                def out_fn(ih, l=l):
                    return hpatch[l // 2][64 * (l % 2):64 * (l % 2) + 64,
                                          ih * 512:(ih + 1) * 512]

                _attention_head(nc, pools, whO_fn, w1rows[l][:],
                                wh2_fn, mT_sb, out_fn)

            # ---- stage G: exchange ----
            patch_d = dram.tile([2, 128, QC], F16)
            gath_d = dram.tile([NCORES, 2, 128, QC], F16, addr_space="Shared")
            pw1_d = dram.tile([1, QC], F32)
            w1l2_d = dram.tile([1, QC], F32)

            for hh in range(2):
                nc.sync.dma_start(patch_d[hh], hpatch[hh][:])
            nc.gpsimd.collective_compute(
                "AllGather", OP.bypass,
                replica_groups=[list(range(NCORES))],
                ins=[patch_d.opt()], outs=[gath_d.opt()])

            # partial wh1_L2 over this core's 256 h-feature rows
            psw2 = [psumF.tile([1, 512], F32, tag="fps", name=f"psw2{ih}")
                    for ih in range(2)]
            for ih in range(2):
                for kc in range(2):
                    nc.tensor.matmul(
                        psw2[ih][:], lhsT=g1l2m_sb[:, kc:kc + 1],
                        rhs=hpatch[kc][:, ih * 512:(ih + 1) * 512],
                        start=(kc == 0), stop=(kc == 1))
            pwrow = small.tile([1, QC], F32, tag="pwrow")
            for ih in range(2):
                nc.vector.tensor_copy(pwrow[:, ih * 512:(ih + 1) * 512],
                                      psw2[ih][:])
            nc.sync.dma_start(pw1_d[:], pwrow[:])
            nc.gpsimd.collective_compute(
                "AllReduce", OP.add,
                replica_groups=[[0, 4], [1, 5], [2, 6], [3, 7]],
                ins=[pw1_d.opt()], outs=[w1l2_d.opt()])
            nc.sync.dma_start(w1l2row[:], w1l2_d[:])

            # ---- stage F2: layer-2 feature matmul (streams hT from gather) ----
            whO2_r = whO2.rearrange("p (n c) -> p n c", c=65)
            if True:
                for nt in range(NT):
                    psF2 = psumF.tile([128, 66], F32, tag="fps")
                    for ft in range(4):
                        hgp, hh = ft // 2, ft % 2
                        cp = 4 * hgp + nt // 8
                        col = (nt % 8) * 128
                        hs = stream.tile([128, 128], F16, tag="hs", bufs=8)
                        nc.gpsimd.dma_start(hs[:],
                                          gath_d[cp, hh, :, col:col + 128])
                        nc.tensor.matmul(psF2[:], lhsT=hs[:],
                                         rhs=w2cat_sb[:, ft * 66:(ft + 1) * 66],
                                         start=(ft == 0), stop=(ft == 3))
                    nc.vector.tensor_copy(whO2_r[:, nt, 0:64], psF2[:, 0:64])
                    nc.vector.tensor_copy(wh2L2[:, nt:nt + 1], psF2[:, 65:66])

            # ---- stage A2: layer-2 attention (redundant across hg pairs) ----
            def whO2_fn(jt):
                return whO2[:, jt * 65:jt * 65 + 65]

            def wh2L2_fn(jt):
                if jt is None:
                    return wh2L2[:, :]
                return wh2L2[:, jt:jt + 1]

            def out2_fn(ih):
                return oe[:, ih * 512:(ih + 1) * 512]

            _attention_head(nc, pools, whO2_fn, w1l2row[:],
                            wh2L2_fn, mT_sb, out2_fn)

            # ---- log_softmax over classes (partition dim, via PE colsum) ----
            nc.scalar.activation(e2[:], oe[:], AF.Exp)
            for ih in range(2):
                psls = psumF.tile([1, 512], F32, tag="fps")
                nc.tensor.matmul(psls[:], lhsT=ones64[:],
                                 rhs=e2[:, ih * 512:(ih + 1) * 512],
                                 start=True, stop=True)
                nc.scalar.activation(lse[:, ih * 512:(ih + 1) * 512],
                                     psls[:], AF.Ln)
            _bcast_via_dram(nc, dram, lsebc[:], lse[:], 64, QC, F32, "bc_lse")
            nc.vector.tensor_sub(e2[:], oe[:], lsebc[:])
            nc.sync.dma_start(out[:, :], e2[:])

    nfix = _legalize_dma_waits(nc)
    if os.environ.get("GAT_DEBUG"):
        print(f"legalized {nfix} multi-wait DMAs")
    return nc


_PROGRAM = None


def _get_program():
    global _PROGRAM
    if _PROGRAM is None:
        _PROGRAM = build_program()
    return _PROGRAM


def make_in_maps(x, adj, W, a1, a2, W2, a21, a22):
    f16 = np.float16
    x = np.asarray(x, np.float32)
    adj = np.asarray(adj)
    W = np.asarray(W, np.float32)
    a1 = np.asarray(a1, np.float32)
    a2 = np.asarray(a2, np.float32)
    W2 = np.asarray(W2, np.float32)
    a21 = np.asarray(a21, np.float32)
    a22 = np.asarray(a22, np.float32)

    xT16 = np.ascontiguousarray(x.T).astype(f16)
    mask = (adj > 0).astype(f16)

    g1 = np.einsum("hfk,hf->hk", W, a1).astype(np.float32)   # [H, FIN]
    g2 = np.einsum("hfk,hf->hk", W, a2).astype(np.float32)
    g1l2 = (W2.T @ a21).astype(np.float32)                   # [FIN]
    g2l2 = (W2.T @ a22).astype(np.float32)
    w2cat = np.concatenate(
        [W2.T, g1l2[:, None], g2l2[:, None]], axis=1).astype(f16)  # [512, 66]

    in_maps = []
    for c in range(NCORES):
        rg, hg = c % 4, c // 4
        q0 = QC * rg
        heads = range(HPC * hg, HPC * hg + HPC)
        wcat = np.concatenate(
            [np.concatenate([W[h].T, g1[h][:, None], g2[h][:, None]], axis=1)
             for h in heads], axis=1).astype(f16)             # [512, 4*66]
        gbm = np.stack([g1[h] for h in heads], axis=1).astype(f16)  # [512, 4]
        g1l2m = g1l2[256 * hg:256 * (hg + 1)].reshape(2, 128).T.astype(f16)
        in_maps.append({
            "xT": xT16,
            "xTq": np.ascontiguousarray(x[q0:q0 + QC].T).astype(f16),
            "mT": np.ascontiguousarray(mask[q0:q0 + QC].T),
            "wcat": wcat,
            "gb": gbm,
            "w2cat": w2cat,
            "g1l2m": np.ascontiguousarray(g1l2m),
        })
    return in_maps


def kernel(x, adj, W, a1, a2, W2, a21, a22):
    from concourse.bass_utils import run_bass_kernel_spmd

    nc = _get_program()
    in_maps = make_in_maps(x, adj, W, a1, a2, W2, a21, a22)
    trace = bool(int(os.environ.get("GAT_TRACE", "0")))
    res = run_bass_kernel_spmd(nc, in_maps, core_ids=list(range(NCORES)),
                               trace=trace)
    if trace and res.exec_time_ns is not None:
        print(f"HW exec time: {res.exec_time_ns} ns")
        kernel._last_exec_ns = res.exec_time_ns
    kernel._last_results = res

    full = np.empty((N, NCLS), np.float32)
    for rg in range(4):
        full[QC * rg:QC * (rg + 1), :] = res.results[rg]["out"].T
    return full
